# revision 9
# baseline (speedup 1.0000x reference)
"""CenterLoss Trainium2 kernel (8 NeuronCores, data-parallel over batch).

loss = clip(cosine_dist(features, centers) * onehot(targets), EPS, MAXV).sum() / B

The onehot mask keeps exactly one column per row, so the (B, C) distance
matrix is never needed: each row only requires
    d_b = 1 - <f_b, c_{t_b}> / (||f_b|| ||c_{t_b}||)
The remaining B*(C-1) masked zeros clip to EPS, contributing the exact
constant (C-1)*EPS to the loss.

Sharding strategy (host side): batch is split across the 8 cores; centers
are sharded BY TARGET INDEX — each core receives exactly the 512 center
rows its batch shard points at, interleaved with the feature rows so each
128-row block is one dense 4KB-per-partition DMA. Compute runs in bf16
(f32 accumulation), which keeps the loss within ~1e-5 relative.

Per core (batch shard of 512 rows = 4 blocks of 128):
  - 4 pipelined HWDGE DMAs, one [f_j | g_j] block each (256 KB bf16)
  - fused multiply+row-reduce: DVE does <f,g> and <f,f>, GPSIMD does <g,g>
  - tail: d = max(1 - fc/sqrt(ff*gg), EPS), row-sum -> [128,1] lane sums
    (the 1e12 upper clip is a no-op: d = 1 - cos <= 2 by construction)
  - host folds the 8x128 partial sums (f64) and adds (C-1)*EPS.
"""

import sys

for _p in ("/opt/trn_rl_repo", "/opt/pypackages"):
    if _p not in sys.path:
        sys.path.insert(0, _p)

import ml_dtypes
import numpy as np

B = 4096
D = 512
C = 10000
NCORES = 8
BS = B // NCORES  # 512 rows per core
JBLK = BS // 128  # 4 partition blocks
EPS = 1e-12
MAXV = 1e12

_cached_nc = None


def _build():
    global _cached_nc
    if _cached_nc is not None:
        return _cached_nc

    from concourse import bacc, mybir
    from concourse.tile import TileContext

    f32 = mybir.dt.float32
    bf16 = mybir.dt.bfloat16
    mult = mybir.AluOpType.mult

    nc = bacc.Bacc()
    fg = nc.declare_dram_parameter("fg", [JBLK, 128, 2, D], bf16, isOutput=False)
    outp = nc.declare_dram_parameter("out", [128, 1], f32, isOutput=True)

    with TileContext(nc) as tc:
        with (
            tc.tile_pool(name="main", bufs=1) as pool,
            tc.tile_pool(name="blocks", bufs=JBLK) as blocks,
            tc.tile_pool(name="junk", bufs=2) as junk,
        ):
            # Pin the ACT table to the 'sqrt_and_others' set (contains both
            # sqrt and square) via a dummy sqrt as ACT's first instruction —
            # otherwise the table-load pass loads a square-only set first and
            # reloads a sqrt set mid-kernel (1.28us on the critical path).
            dummy = pool.tile([128, 1], f32)
            nc.gpsimd.memset(dummy[:], 1.0)
            nc.scalar.activation(
                out=dummy[:], in_=dummy[:], func=mybir.ActivationFunctionType.Sqrt
            )

            fc = pool.tile([128, JBLK], f32)
            ff = pool.tile([128, JBLK], f32)
            gg = pool.tile([128, JBLK], f32)
            for j in range(JBLK):
                t = blocks.tile([128, 2, D], bf16, tag="blk")
                # Split descriptor generation across the SP and ACT HWDGE
                # sequencers so the four input DMAs don't serialize on one.
                dma_eng = nc.sync if j % 2 == 0 else nc.scalar
                dma_eng.dma_start(out=t[:], in_=fg[j, :, :, :])
                f_j = t[:, 0, :]
                g_j = t[:, 1, :]
                prod = junk.tile([128, D], bf16, tag="prod")
                nc.vector.scalar_tensor_tensor(
                    out=prod[:],
                    in0=f_j,
                    scalar=1.0,
                    in1=g_j,
                    op0=mult,
                    op1=mult,
                    accum_out=fc[:, j : j + 1],
                )
                sqf = junk.tile([128, D], bf16, tag="sqf")
                nc.vector.scalar_tensor_tensor(
                    out=sqf[:],
                    in0=f_j,
                    scalar=1.0,
                    in1=f_j,
                    op0=mult,
                    op1=mult,
                    accum_out=ff[:, j : j + 1],
                )
                sqg = junk.tile([128, D], bf16, tag="sqg")
                nc.scalar.activation(
                    out=sqg[:],
                    in_=g_j,
                    func=mybir.ActivationFunctionType.Square,
                    accum_out=gg[:, j : j + 1],
                )

            # d = max(1 - fc / sqrt(ff*gg), EPS), then row-sum.
            t2 = pool.tile([128, JBLK], f32)
            nc.vector.tensor_tensor(out=t2[:], in0=ff[:], in1=gg[:], op=mult)
            s = pool.tile([128, JBLK], f32)
            nc.scalar.activation(
                out=s[:], in_=t2[:], func=mybir.ActivationFunctionType.Sqrt
            )
            r = pool.tile([128, JBLK], f32)
            nc.vector.reciprocal(out=r[:], in_=s[:])
            negm = pool.tile([128, JBLK], f32)
            nc.vector.scalar_tensor_tensor(
                out=negm[:],
                in0=fc[:],
                scalar=-1.0,
                op0=mult,
                in1=r[:],
                op1=mult,
            )
            dc = pool.tile([128, JBLK], f32)
            nc.vector.tensor_scalar(
                out=dc[:],
                in0=negm[:],
                scalar1=1.0,
                scalar2=EPS,
                op0=mybir.AluOpType.add,
                op1=mybir.AluOpType.max,
            )
            dsum = pool.tile([128, 1], f32)
            nc.vector.reduce_sum(dsum[:], dc[:], axis=mybir.AxisListType.X)
            # SWDGE (gpsimd) for the result DMA: its completion semaphore is
            # updated in-ring right after the data descriptors, while the
            # HWDGE path was observed to post the final completion ~6.5us
            # after the transfer finished.
            nc.gpsimd.dma_start(out=outp[:, :], in_=dsum[:])

    nc.compile()
    _cached_nc = nc
    return nc


def _make_in_maps(features, centers, targets):
    features = np.ascontiguousarray(features, dtype=np.float32)
    centers = np.ascontiguousarray(centers, dtype=np.float32)
    targets = np.asarray(targets)
    gathered = centers[targets]  # (B, D): center row for each batch row
    in_maps = []
    for c in range(NCORES):
        lo, hi = c * BS, (c + 1) * BS
        fg = np.empty((JBLK, 128, 2, D), dtype=ml_dtypes.bfloat16)
        fg[:, :, 0] = features[lo:hi].reshape(JBLK, 128, D)
        fg[:, :, 1] = gathered[lo:hi].reshape(JBLK, 128, D)
        in_maps.append({"fg": fg})
    return in_maps


def _combine(partials):
    total = float(np.sum(np.asarray(partials, dtype=np.float64)))
    return np.float32(total / B + (C - 1) * EPS)


def _run(features, centers, targets, **spmd_kwargs):
    from concourse.bass_utils import run_bass_kernel_spmd

    nc = _build()
    in_maps = _make_in_maps(features, centers, targets)
    out = run_bass_kernel_spmd(nc, in_maps, core_ids=list(range(NCORES)), **spmd_kwargs)
    partials = [out.results[c]["out"].astype(np.float64).sum() for c in range(NCORES)]
    return _combine(partials), out


def kernel(features, centers, targets):
    loss, _ = _run(features, centers, targets)
    return loss


# revision 10
# speedup vs baseline: 1.2165x; 1.2165x over previous
"""CenterLoss Trainium2 kernel (8 NeuronCores, data-parallel over batch).

loss = clip(cosine_dist(features, centers) * onehot(targets), EPS, MAXV).sum() / B

The onehot mask keeps exactly one column per row, so the (B, C) distance
matrix is never needed: each row only requires
    d_b = 1 - <f_b, c_{t_b}> / (||f_b|| ||c_{t_b}||)
The remaining B*(C-1) masked zeros clip to EPS, contributing the exact
constant (C-1)*EPS to the loss.

Sharding strategy (host side): batch is split across the 8 cores; centers
are sharded BY TARGET INDEX — each core receives exactly the 512 center
rows its batch shard points at, interleaved with the feature rows so each
128-row block is one dense 4KB-per-partition DMA. Compute runs in bf16
(f32 accumulation), which keeps the loss within ~1e-5 relative.

Per core (batch shard of 512 rows = 4 blocks of 128):
  - 4 pipelined HWDGE DMAs, one [f_j | g_j] block each (256 KB bf16)
  - fused multiply+row-reduce: DVE does <f,g> and <f,f>, GPSIMD does <g,g>
  - tail: d = max(1 - fc/sqrt(ff*gg), EPS), row-sum -> [128,1] lane sums
    (the 1e12 upper clip is a no-op: d = 1 - cos <= 2 by construction)
  - host folds the 8x128 partial sums (f64) and adds (C-1)*EPS.
"""

import sys

for _p in ("/opt/trn_rl_repo", "/opt/pypackages"):
    if _p not in sys.path:
        sys.path.insert(0, _p)

import ml_dtypes
import numpy as np

B = 4096
D = 512
C = 10000
NCORES = 8
BS = B // NCORES  # 512 rows per core
JBLK = BS // 128  # 4 partition blocks
EPS = 1e-12
MAXV = 1e12

_cached_nc = None


def _build():
    global _cached_nc
    if _cached_nc is not None:
        return _cached_nc

    from concourse import bacc, mybir
    from concourse.tile import TileContext

    f32 = mybir.dt.float32
    bf16 = mybir.dt.bfloat16
    mult = mybir.AluOpType.mult

    nc = bacc.Bacc()
    fg = nc.declare_dram_parameter("fg", [JBLK, 128, 2, D], bf16, isOutput=False)
    outp = nc.declare_dram_parameter("out", [128, 128], f32, isOutput=True)

    with TileContext(nc) as tc:
        with (
            tc.tile_pool(name="main", bufs=1) as pool,
            tc.tile_pool(name="blocks", bufs=JBLK) as blocks,
            tc.tile_pool(name="junk", bufs=2) as junk,
        ):
            # Pin the ACT table to the 'sqrt_and_others' set (contains both
            # sqrt and square) via a dummy sqrt as ACT's first instruction —
            # otherwise the table-load pass loads a square-only set first and
            # reloads a sqrt set mid-kernel (1.28us on the critical path).
            dummy = pool.tile([128, 1], f32)
            nc.gpsimd.memset(dummy[:], 1.0)
            nc.scalar.activation(
                out=dummy[:], in_=dummy[:], func=mybir.ActivationFunctionType.Sqrt
            )

            fc = pool.tile([128, JBLK], f32)
            ff = pool.tile([128, JBLK], f32)
            gg = pool.tile([128, JBLK], f32)
            for j in range(JBLK):
                t = blocks.tile([128, 2, D], bf16, tag="blk")
                # Split descriptor generation across the SP and ACT HWDGE
                # sequencers so the four input DMAs don't serialize on one.
                dma_eng = nc.sync if j % 2 == 0 else nc.scalar
                dma_eng.dma_start(out=t[:], in_=fg[j, :, :, :])
                f_j = t[:, 0, :]
                g_j = t[:, 1, :]
                prod = junk.tile([128, D], bf16, tag="prod")
                nc.vector.scalar_tensor_tensor(
                    out=prod[:],
                    in0=f_j,
                    scalar=1.0,
                    in1=g_j,
                    op0=mult,
                    op1=mult,
                    accum_out=fc[:, j : j + 1],
                )
                sqf = junk.tile([128, D], bf16, tag="sqf")
                if j < 3:  # ff block 3 runs on ACT (engine balance)
                    nc.vector.scalar_tensor_tensor(
                        out=sqf[:],
                        in0=f_j,
                        scalar=1.0,
                        in1=f_j,
                        op0=mult,
                        op1=mult,
                        accum_out=ff[:, j : j + 1],
                    )
                else:
                    nc.scalar.activation(
                        out=sqf[:],
                        in_=f_j,
                        func=mybir.ActivationFunctionType.Square,
                        accum_out=ff[:, j : j + 1],
                    )
                sqg = junk.tile([128, D], bf16, tag="sqg")
                nc.scalar.activation(
                    out=sqg[:],
                    in_=g_j,
                    func=mybir.ActivationFunctionType.Square,
                    accum_out=gg[:, j : j + 1],
                )

            # d = max(1 - fc / sqrt(ff*gg), EPS), then row-sum.
            t2 = pool.tile([128, JBLK], f32)
            nc.vector.tensor_tensor(out=t2[:], in0=ff[:], in1=gg[:], op=mult)
            s = pool.tile([128, JBLK], f32)
            nc.scalar.activation(
                out=s[:], in_=t2[:], func=mybir.ActivationFunctionType.Sqrt
            )
            r = pool.tile([128, JBLK], f32)
            nc.vector.reciprocal(out=r[:], in_=s[:])
            negm = pool.tile([128, JBLK], f32)
            nc.vector.scalar_tensor_tensor(
                out=negm[:],
                in0=fc[:],
                scalar=-1.0,
                op0=mult,
                in1=r[:],
                op1=mult,
            )
            dc = pool.tile([128, JBLK], f32)
            nc.vector.tensor_scalar(
                out=dc[:],
                in0=negm[:],
                scalar1=1.0,
                scalar2=EPS,
                op0=mybir.AluOpType.add,
                op1=mybir.AluOpType.max,
            )
            # Pad the result DMA to 512B descriptors: <=256B descriptors go
            # through the SDMA packetization path whose completion semaphore
            # only posts after an idle-flush (~6.6us observed on the final
            # DMA of the kernel). 512B/partition bypasses it.
            dsum = pool.tile([128, 128], f32)
            nc.gpsimd.memset(dsum[:], 0.0)
            nc.vector.reduce_sum(dsum[:, 0:1], dc[:], axis=mybir.AxisListType.X)
            # SWDGE (gpsimd) for the result DMA: its completion semaphore is
            # updated in-ring right after the data descriptors, while the
            # HWDGE path was observed to post the final completion ~6.5us
            # after the transfer finished.
            nc.gpsimd.dma_start(out=outp[:, :], in_=dsum[:])

    nc.compile()
    _cached_nc = nc
    return nc


def _make_in_maps(features, centers, targets):
    features = np.ascontiguousarray(features, dtype=np.float32)
    centers = np.ascontiguousarray(centers, dtype=np.float32)
    targets = np.asarray(targets)
    gathered = centers[targets]  # (B, D): center row for each batch row
    in_maps = []
    for c in range(NCORES):
        lo, hi = c * BS, (c + 1) * BS
        fg = np.empty((JBLK, 128, 2, D), dtype=ml_dtypes.bfloat16)
        fg[:, :, 0] = features[lo:hi].reshape(JBLK, 128, D)
        fg[:, :, 1] = gathered[lo:hi].reshape(JBLK, 128, D)
        in_maps.append({"fg": fg})
    return in_maps


def _combine(partials):
    total = float(np.sum(np.asarray(partials, dtype=np.float64)))
    return np.float32(total / B + (C - 1) * EPS)


def _run(features, centers, targets, **spmd_kwargs):
    from concourse.bass_utils import run_bass_kernel_spmd

    nc = _build()
    in_maps = _make_in_maps(features, centers, targets)
    out = run_bass_kernel_spmd(nc, in_maps, core_ids=list(range(NCORES)), **spmd_kwargs)
    partials = [out.results[c]["out"][:, 0].astype(np.float64).sum() for c in range(NCORES)]
    return _combine(partials), out


def kernel(features, centers, targets):
    loss, _ = _run(features, centers, targets)
    return loss


# revision 12
# speedup vs baseline: 1.3222x; 1.0869x over previous
"""CenterLoss Trainium2 kernel (8 NeuronCores, data-parallel over batch).

loss = clip(cosine_dist(features, centers) * onehot(targets), EPS, MAXV).sum() / B

The onehot mask keeps exactly one column per row, so the (B, C) distance
matrix is never needed: each row only requires
    d_b = 1 - <f_b, c_{t_b}> / (||f_b|| ||c_{t_b}||)
The remaining B*(C-1) masked zeros clip to EPS, contributing the exact
constant (C-1)*EPS to the loss.

Sharding strategy (host side): batch is split across the 8 cores; centers
are sharded BY TARGET INDEX - each core receives exactly the 512 center
rows its batch shard points at, interleaved with the feature rows so each
128-row block is one dense 2KB-per-partition DMA. Compute runs in bf16
(f32 accumulation), which keeps the loss within ~4e-6 relative.

Per core (batch shard of 512 rows = 4 blocks of 128), raw bacc engine
blocks with manual semaphores (no Tile framework):
  - 4 pipelined HWDGE input DMAs, descgen split across SP + ACT sequencers
  - fused multiply+row-accumulate: DVE does <f,g> and <f,f> (7 ops),
    ACT does <g,g> plus one <f,f> (5 ops) - balanced engine split
  - ACT table pinned to the sqrt_and_others set via a dummy first sqrt
    (one hidden 1.28us table load instead of two)
  - tail: d = max(1 - fc/sqrt(ff*gg), EPS), row-sum
    (the 1e12 upper clip is a no-op: d = 1 - cos <= 2 by construction)
  - result DMA padded to 512B/partition descriptors (<=256B descriptors
    hit an SDMA packetization idle-flush that posts the completion
    semaphore ~6.6us late), issued from the idle SP HWDGE sequencer
  - host folds the 8x128 partial sums (f64) and adds (C-1)*EPS.

Measured (neuron-profile, whole NEFF): ~20.2us exec, rel err 3.2e-06.
"""

import sys

for _p in ("/opt/trn_rl_repo", "/opt/pypackages"):
    if _p not in sys.path:
        sys.path.insert(0, _p)

import ml_dtypes
import numpy as np

B = 4096
D = 512
C = 10000
NCORES = 8
BS = B // NCORES
JBLK = BS // 128
EPS = 1e-12
MAXV = 1e12

_cached_nc = None


def _build():
    global _cached_nc
    if _cached_nc is not None:
        return _cached_nc

    from concourse import bacc, mybir

    f32 = mybir.dt.float32
    bf16 = mybir.dt.bfloat16
    mult = mybir.AluOpType.mult

    nc = bacc.Bacc()
    fg = nc.declare_dram_parameter("fg", [JBLK, 128, 2, D], bf16, isOutput=False)
    outp = nc.declare_dram_parameter("out", [128, 128], f32, isOutput=True)

    from contextlib import ExitStack

    with ExitStack() as st:
        e = st.enter_context
        t0 = e(nc.sbuf_tensor("t0", [128, 2, D], bf16))
        t1 = e(nc.sbuf_tensor("t1", [128, 2, D], bf16))
        t2b = e(nc.sbuf_tensor("t2b", [128, 2, D], bf16))
        t3 = e(nc.sbuf_tensor("t3", [128, 2, D], bf16))
        prods = [e(nc.sbuf_tensor(f"prod{j}", [128, D], bf16)) for j in range(JBLK)]
        sqfs = [e(nc.sbuf_tensor(f"sqf{j}", [128, D], bf16)) for j in range(JBLK)]
        sqgs = [e(nc.sbuf_tensor(f"sqg{j}", [128, D], bf16)) for j in range(JBLK)]
        fc = e(nc.sbuf_tensor("fc", [128, JBLK], f32))
        ff = e(nc.sbuf_tensor("ff", [128, JBLK], f32))
        gg = e(nc.sbuf_tensor("gg", [128, JBLK], f32))
        t2 = e(nc.sbuf_tensor("t2", [128, JBLK], f32))
        s = e(nc.sbuf_tensor("s", [128, JBLK], f32))
        r = e(nc.sbuf_tensor("r", [128, JBLK], f32))
        negm = e(nc.sbuf_tensor("negm", [128, JBLK], f32))
        dc = e(nc.sbuf_tensor("dc", [128, JBLK], f32))
        dsum = e(nc.sbuf_tensor("dsum", [128, 128], f32))
        dummy = e(nc.sbuf_tensor("dpin", [128, 1], f32))
        dma0 = e(nc.semaphore("dma0"))
        dma1 = e(nc.semaphore("dma1"))
        dma2 = e(nc.semaphore("dma2"))
        dma3 = e(nc.semaphore("dma3"))
        dmao = e(nc.semaphore("dmao"))
        sv = e(nc.semaphore("sv"))
        sp = e(nc.semaphore("sp"))
        sa = e(nc.semaphore("sa"))
        block = e(nc.Block())

        tiles = [t0, t1, t2b, t3]
        dsems = [dma0, dma1, dma2, dma3]

        @block.sync
        def _(sync):
            for j in (0, 2):
                sync.dma_start(out=tiles[j][:], in_=fg[j, :, :, :]).then_inc(
                    dsems[j], 16
                )
            # HWDGE result DMA from the otherwise-idle SP sequencer (descgen
            # 625ns vs the SWDGE Q7 path's ~1.6us). 512B/partition descriptors
            # keep the completion semaphore off the packetization idle-flush.
            sync.wait_ge(sv, 12)
            sync.dma_start(out=outp[:, :], in_=dsum[:]).then_inc(dmao, 16)
            sync.wait_ge(dmao, 16)

        @block.vector
        def _(vector):
            for j in range(JBLK):
                vector.wait_ge(dsems[j], 16)
                vector.scalar_tensor_tensor(
                    out=prods[j][:],
                    in0=tiles[j][:, 0, :],
                    scalar=1.0,
                    in1=tiles[j][:, 1, :],
                    op0=mult,
                    op1=mult,
                    accum_out=fc[:, j : j + 1],
                ).then_inc(sv, 1)
                if j < 3:  # ff block 3 runs on ACT (engine balance)
                    vector.scalar_tensor_tensor(
                        out=sqfs[j][:],
                        in0=tiles[j][:, 0, :],
                        scalar=1.0,
                        in1=tiles[j][:, 0, :],
                        op0=mult,
                        op1=mult,
                        accum_out=ff[:, j : j + 1],
                    ).then_inc(sv, 1)
            vector.wait_ge(sv, 7)  # own-pipeline drain before reading ff/fc
            vector.wait_ge(sa, 6)  # 4 squares + ff3 + dummy sqrt
            vector.tensor_tensor(out=t2[:], in0=ff[:], in1=gg[:], op=mult).then_inc(
                sv, 1
            )
            vector.wait_ge(sa, 7)  # real sqrt done (dummy+4gg+ff3+sqrt)
            vector.reciprocal(out=r[:], in_=s[:]).then_inc(sv, 1)
            vector.wait_ge(sv, 9)
            vector.scalar_tensor_tensor(
                out=negm[:],
                in0=fc[:],
                scalar=-1.0,
                op0=mult,
                in1=r[:],
                op1=mult,
            ).then_inc(sv, 1)
            vector.wait_ge(sv, 10)
            vector.tensor_scalar(
                out=dc[:],
                in0=negm[:],
                scalar1=1.0,
                scalar2=EPS,
                op0=mybir.AluOpType.add,
                op1=mybir.AluOpType.max,
            ).then_inc(sv, 1)
            vector.wait_ge(sv, 11)
            vector.wait_ge(sp, 2)  # dsum pad memset done before writing col 0
            vector.tensor_reduce(
                dsum[:, 0:1], dc[:], axis=mybir.AxisListType.X, op=mybir.AluOpType.add
            ).then_inc(sv, 1)

        @block.gpsimd
        def _(gpsimd):
            gpsimd.memset(dummy[:], 1.0).then_inc(sp, 1)
            # Pad the result DMA to 512B descriptors: <=256B descriptors go
            # through the SDMA packetization path whose completion semaphore
            # only posts after an idle-flush (~6.6us observed on the final
            # DMA of the kernel). 512B/partition bypasses it.
            gpsimd.memset(dsum[:], 0.0).then_inc(sp, 1)

        @block.scalar
        def _(scalar):
            # Input DMAs for blocks 1,3 via the ACT HWDGE sequencer so their
            # descriptor generation overlaps the SP sequencer's blocks 0,2.
            for j in (1, 3):
                scalar.dma_start(out=tiles[j][:], in_=fg[j, :, :, :]).then_inc(
                    dsems[j], 16
                )
            # Dummy sqrt first: pins the ACT table to the sqrt_and_others set
            # (which also contains square) so only one table load happens.
            scalar.wait_ge(sp, 1)
            scalar.activation(
                out=dummy[:], in_=dummy[:], func=mybir.ActivationFunctionType.Sqrt
            ).then_inc(sa, 1)
            for j in range(JBLK):
                scalar.wait_ge(dsems[j], 16)
                scalar.activation(
                    out=sqgs[j][:],
                    in_=tiles[j][:, 1, :],
                    func=mybir.ActivationFunctionType.Square,
                    accum_out=gg[:, j : j + 1],
                ).then_inc(sa, 1)
            scalar.activation(
                out=sqfs[3][:],
                in_=tiles[3][:, 0, :],
                func=mybir.ActivationFunctionType.Square,
                accum_out=ff[:, 3:4],
            ).then_inc(sa, 1)
            scalar.wait_ge(sv, 8)  # 7 DVE block ops + t2
            scalar.activation(
                out=s[:], in_=t2[:], func=mybir.ActivationFunctionType.Sqrt
            ).then_inc(sa, 1)

    nc.compile()
    _cached_nc = nc
    return nc


def _make_in_maps(features, centers, targets):
    features = np.ascontiguousarray(features, dtype=np.float32)
    centers = np.ascontiguousarray(centers, dtype=np.float32)
    targets = np.asarray(targets)
    gathered = centers[targets]
    in_maps = []
    for c in range(NCORES):
        lo, hi = c * BS, (c + 1) * BS
        fg = np.empty((JBLK, 128, 2, D), dtype=ml_dtypes.bfloat16)
        fg[:, :, 0] = features[lo:hi].reshape(JBLK, 128, D)
        fg[:, :, 1] = gathered[lo:hi].reshape(JBLK, 128, D)
        in_maps.append({"fg": fg})
    return in_maps


def _combine(partials):
    total = float(np.sum(np.asarray(partials, dtype=np.float64)))
    return np.float32(total / B + (C - 1) * EPS)


def _run(features, centers, targets, **spmd_kwargs):
    from concourse.bass_utils import run_bass_kernel_spmd

    nc = _build()
    in_maps = _make_in_maps(features, centers, targets)
    out = run_bass_kernel_spmd(nc, in_maps, core_ids=list(range(NCORES)), **spmd_kwargs)
    partials = [out.results[c]["out"][:, 0].astype(np.float64).sum() for c in range(NCORES)]
    return _combine(partials), out


def kernel(features, centers, targets):
    loss, _ = _run(features, centers, targets)
    return loss
